# revision 3
# baseline (speedup 1.0000x reference)
import hashlib
from functools import lru_cache, partial

import numpy as np
import jax
import jax.numpy as jnp

# nn_LocalMultiHeadChannelAttention: B=16, C=512, R=32, PS=3, HN=8, D=128,
# input spatial H=W=96. Sharded data-parallel over batch B across 8 cores
# (2 batches/core); all params replicated. No collectives needed.
B, C, R, PS, HN, D = 16, 512, 32, 3, 8, 128
NORM_C = 0.5
NCORES = 8
PARAM_NAMES = ("Wqk", "bqk", "Wp", "bp", "Wv", "bv")


def _to_heads(p, b):
    # [b,C,R,R] -> [b,HN,C,D] via the reference's reshape/permute chain
    t = p.reshape(b, R * R, C).transpose(0, 2, 1)
    return t.reshape(b, C, HN, D).transpose(0, 2, 1, 3)


def _shard_body(x, Wqk, bqk, Wp, bp, Wv, bv, wscale):
    b = x.shape[0]
    xr = x.reshape(b, C, R, PS, R, PS)
    q_pool = xr.mean(axis=(3, 5))            # [b, C, R, R]
    k_pool = xr.max(axis=(3, 5))

    q = jnp.einsum('bhcd,hed->bhce', _to_heads(q_pool, b), Wqk) + bqk[None, :, None, :]
    k = jnp.einsum('bhcd,hed->bhce', _to_heads(k_pool, b), Wqk) + bqk[None, :, None, :]

    # 1x1 conv commutes with avg-pool: avg_pool3(Wv@x + bv) == Wv@q_pool + bv
    v_conv = jnp.einsum('bchw,oc->bohw', q_pool, Wv) + bv[None, :, None, None]
    v = _to_heads(v_conv, b)

    scores = jnp.einsum('bhcd,bhed->bhce', q, k)          # [b,HN,C,C]
    p = jax.nn.sigmoid(scores.mean(axis=-1) @ Wp.T + bp)  # [b,HN,C]
    norm_scores = scores / jnp.power(jnp.float32(D), NORM_C + p[..., None])
    w = jax.nn.softmax(norm_scores, axis=-1)
    attn = jnp.einsum('bhce,bhed->bhcd', w, v)

    attn = attn.transpose(0, 2, 1, 3).reshape(b, C, R * R)
    attn = attn.transpose(0, 2, 1).reshape(b, R, R, C)
    resid = q_pool.reshape(b, R * R, C).reshape(b, R, R, C)
    return resid + attn * wscale


@lru_cache(maxsize=4)
def _build(wscale):
    return jax.pmap(partial(_shard_body, wscale=np.float32(wscale)),
                    in_axes=0, devices=jax.devices()[:NCORES])


_param_cache = {}


def _params_on_device(params):
    key = hashlib.md5(b"".join(p.tobytes() for p in params)).hexdigest()
    if key not in _param_cache:
        devs = jax.devices()[:NCORES]
        _param_cache.clear()
        _param_cache[key] = tuple(jax.device_put_replicated(p, devs)
                                  for p in params)
    return _param_cache[key]


def kernel(x, Wqk, bqk, Wp, bp, Wv, bv, weight):
    x = np.asarray(x, dtype=np.float32)
    wscale = float(1 + int(np.asarray(weight)))
    params = tuple(np.asarray(t, dtype=np.float32) for t in (Wqk, bqk, Wp, bp, Wv, bv))

    xs = x.reshape(NCORES, B // NCORES, C, PS * R, PS * R)
    xs_d = jax.device_put_sharded(list(xs), jax.devices()[:NCORES])
    out = _build(wscale)(xs_d, *_params_on_device(params))
    return np.asarray(out).reshape(B, R, R, C).astype(np.float32)
